# revision 110
# baseline (speedup 1.0000x reference)
"""Multi-head causal attention (RoPE + QK-RMSNorm) on 8 TRN2 NeuronCores.

Sharding: data parallel on batch (2) x tensor parallel on heads (4 groups of
2 heads).  core = 4*b + g computes, for batch b, heads [2g, 2g+1]:
  q/k/v projections (E-sliced), qk-rmsnorm + rope, causal attention, and the
  Wo partial product over its E slice.  Host sums the 4 partials per batch.

The big GEMMs (q/k/v projections and the Wo product) run as fp8e4
DoubleRow matmuls with an exact hi+lo mantissa split: every bf16 operand a
is stored as fp8 pair (a_hi, a_lo = a - a_hi), and a 2048-contraction
column block takes 8 hi.hi DoubleRow passes (k-tile pairs) plus 16 per-
k-tile cross passes packing (W_lo, W_hi) against (x_hi, x_lo) — 3/4 the
PE cycles of bf16 at the same accuracy (only lo.lo, ~1e-3 relative, is
dropped).  Inputs are pre-split/prescaled on the host (x by 2^3, W by 2^7);
the q/k rsqrt absorbs the 2^20 on the squares, v is rescaled at its psum
copy, and the attention itself (scores, exp, av) stays bf16.  aT is split
hi/lo on-device (x2^4) for the fp8 Wo product; the 2^11 product scale comes
out at the output stores.

Everything on device runs in the "transposed" orientation:
  qT/kT [e, s], v [s, e], scoresT [sk, sq], out_T [d_out, s]
so no on-chip data transposes are needed; softmax denominators and rms sums
are computed with ones-matmuls on the TensorEngine in *column* layout.
1/sqrt uses a bit-hack seed + 2 Babylonian iterations on DVE so the
ScalarEngine only ever needs the exp table set; the q-normalization scale
path (rsqrt result, row, broadcast) runs in bf16 so the q multiplies hit
DVE 2x mode, and the softmax 1/den column->row scatters alternate the
Pool-SWDGE and Act-HWDGE queues to halve that serial chain.

Scheduling (validated against the TimelineSim cost model): attention is
software-pipelined depth-3 with dens drained before av; projections run
dtp-outer so x tiles and per-pair weight slices are consumed at DMA
arrival order and die early for the next chunk's prefetch (emitted ahead
of store traffic on the SP queue); rope tables ride the Pool SWDGE queue
(chunk 0 on SP) to dodge head-of-line blocking; q/k/v live in per-chunk
tiles to avoid whole-tile cross-chunk waits; Wo bursts inject as delayed
PE filler into later attention groups (leftovers drain at phase end — never
dropped), the last chunk's phase_c runs head-split waves so it isn't gated
on the final softmax-normalization chain, and stores pair two dout blocks
per DMA, alternating SP/ACT queues.  Per-core exec ~250 us (from 308 us
bf16 baseline).
"""

import math

import numpy as np
import ml_dtypes

import concourse.bass as bass
import concourse.tile as tile
from concourse import bacc, mybir
from concourse.bass_utils import run_bass_kernel_spmd

# Problem shapes (hardcoded per instructions).
B = 2
S = 2048
D = 2048
H = 8
HD = 256
HALF = 128
EL = 512          # E columns per core (2 heads)
CH = 512          # sq chunk size
NCH = S // CH     # 4
DT = D // 128     # 16 k-tiles over D
ET = EL // 128    # 4 e-tiles
ST = S // 128     # 16 s-tiles
EPS = 1e-6
N_CORES = 8

BF16 = mybir.dt.bfloat16
F32 = mybir.dt.float32
F8 = mybir.dt.float8e4
I32 = mybir.dt.int32
NBF = ml_dtypes.bfloat16
NF8 = ml_dtypes.float8_e4m3
DR = mybir.MatmulPerfMode.DoubleRow

SQRT_MAGIC = 0x1FBD1DF5     # sqrt(x) seed: (bits(x) >> 1) + MAGIC

# fp8 power-of-2 prescales (keep |values| << 240 = e4m3 max)
SX = 3        # x * 8
SW = 7        # Wq/Wk/Wv * 128
SQV = SX + SW           # q/k/v psum carry 2^10
SA = 4        # aT * 16
SWO = 7       # Wo * 128
SOUT = SA + SWO         # out psum carries 2^11

DTP = DT // 2   # 8 k-tile *pairs* over D
ETP = ET // 2   # 2 e-tile pairs

_CACHE: dict = {}


def _build(reps: int = 1):
    nc = bacc.Bacc("TRN2", target_bir_lowering=False, debug=False,
                   num_devices=N_CORES)

    # fp8 hi/lo split inputs: x8 [dtp, p, dt2, (hi,lo), S],
    # w*8 [dtp, dt2, p, (lo,hi), EL], wo8 [etp, et2, p, (lo,hi), D]
    x8_d = nc.dram_tensor("x8", [DTP, 128, 2, 2, S], F8, kind="ExternalInput").ap()
    wq_d = nc.dram_tensor("wq8", [DTP, 2, 128, 2, EL], F8, kind="ExternalInput").ap()
    wk_d = nc.dram_tensor("wk8", [DTP, 2, 128, 2, EL], F8, kind="ExternalInput").ap()
    wv_d = nc.dram_tensor("wv8", [DTP, 2, 128, 2, EL], F8, kind="ExternalInput").ap()
    wo_d = nc.dram_tensor("wo8", [ETP, 2, 128, 2, D], F8, kind="ExternalInput").ap()
    rtq_d = nc.dram_tensor("rtq", [4, HALF, S], BF16, kind="ExternalInput").ap()
    rtk_d = nc.dram_tensor("rtk", [4, HALF, S], BF16, kind="ExternalInput").ap()
    msk_d = nc.dram_tensor("masks", [4, HALF, CH], BF16, kind="ExternalInput").ap()
    out_d = nc.dram_tensor("outT", [DT, 128, S], BF16, kind="ExternalOutput").ap()

    with tile.TileContext(nc) as tc:
        for _ in range(reps):
            _emit(tc, nc, x8_d, wq_d, wk_d, wv_d, wo_d, rtq_d, rtk_d, msk_d,
                  out_d)
    nc.compile()
    return nc


def _emit(tc, nc, x8_d, wq_d, wk_d, wv_d, wo_d, rtq_d, rtk_d, msk_d, out_d):
    from contextlib import ExitStack
    ctx = ExitStack()
    with ctx:
        persist = ctx.enter_context(tc.tile_pool(name="persist", bufs=1))
        xs_p = ctx.enter_context(tc.tile_pool(name="xs", bufs=9))
        rt_p = ctx.enter_context(tc.tile_pool(name="rt", bufs=1))
        sq_p = ctx.enter_context(tc.tile_pool(name="sq", bufs=4))
        qr_p = ctx.enter_context(tc.tile_pool(name="qr", bufs=9))
        qs_p = ctx.enter_context(tc.tile_pool(name="qs", bufs=4))
        rtmp_p = ctx.enter_context(tc.tile_pool(name="rtmp", bufs=5))
        rb_p = ctx.enter_context(tc.tile_pool(name="rb", bufs=2))
        rd_p = ctx.enter_context(tc.tile_pool(name="rd", bufs=2))
        e_p = ctx.enter_context(tc.tile_pool(name="ep", bufs=7))
        o_p = ctx.enter_context(tc.tile_pool(name="op", bufs=4))
        nr_p = ctx.enter_context(tc.tile_pool(name="nrp", bufs=3))
        at_p = ctx.enter_context(tc.tile_pool(name="atp", bufs=2))

        ps_big = ctx.enter_context(tc.tile_pool(name="psb", bufs=6, space="PSUM"))
        ps_col = ctx.enter_context(tc.tile_pool(name="psc", bufs=2, space="PSUM"))

        # ---- persistent tiles ----
        # wq/wk are split per k-tile pair so the first matmuls only wait on
        # their own 2KB slice, not the whole 16KB weight load.
        wq_t = [persist.tile([128, 2, 2, EL], F8, tag=f"wq{i}",
                              name=f"wq_t{i}") for i in range(DTP)]
        wk_t = [persist.tile([128, 2, 2, EL], F8, tag=f"wk{i}",
                              name=f"wk_t{i}") for i in range(DTP)]
        wv_sb = persist.tile([128, DTP, 2, 2, EL], F8, tag="wv")
        wo_sb = persist.tile([128, ETP, 2, 2, D], F8, tag="wo")
        # per-chunk q/k/v tiles: a single big tile would give phase_b reads
        # a conservative whole-tile dependency on *later* chunks' writes
        qT_t = [persist.tile([128, ET, CH], BF16, tag=f"qT{i}",
                             name=f"qT_t{i}") for i in range(NCH)]
        kT_t = [persist.tile([128, ET, CH], BF16, tag=f"kT{i}",
                             name=f"kT_t{i}") for i in range(NCH)]
        v_t = [persist.tile([128, 4, EL], BF16, tag=f"v{i}",
                            name=f"v_t{i}") for i in range(NCH)]
        msk_sb = persist.tile([128, 4, CH], BF16, tag="msk")
        ones_sb = persist.tile([128, 1], BF16, tag="ones")
        rk_sb = persist.tile([128, 2, ST], F32, tag="rk")   # f_k per (head, sk)
        nc.vector.memset(ones_sb, 1.0)

        # weights go on the Act hwdge queue so they never queue behind the
        # x-tile loads (SP queue); wq/wk stream per k-tile pair.
        W_REARR = "t two p hl e -> p t two hl e"
        WS_REARR = "two p hl e -> p two hl e"
        # wq0 rides the empty-at-start Pool SWDGE queue so its transfer
        # wins the second DMA_ENGINES slot instead of queueing behind x0_1
        nc.gpsimd.dma_start(out=wq_t[0], in_=wq_d[0].rearrange(WS_REARR))
        for i in range(1, DTP):
            nc.scalar.dma_start(out=wq_t[i], in_=wq_d[i].rearrange(WS_REARR))

        def load_wk():
            for i in range(DTP):
                nc.scalar.dma_start(out=wk_t[i],
                                    in_=wk_d[i].rearrange(WS_REARR))

        def load_wv():
            nc.scalar.dma_start(out=wv_sb, in_=wv_d.rearrange(W_REARR))

        def load_rest():
            nc.scalar.dma_start(out=msk_sb, in_=msk_d.rearrange("t p s -> p t s"))
            nc.scalar.dma_start(out=wo_sb, in_=wo_d.rearrange(W_REARR))

        # q/k psums carry 2^SQV; their squares carry 2^(2 SQV), so the eps
        # inside the rsqrt scales along and the rsqrt output absorbs 2^-SQV.
        LN_EPS = float(HD * EPS * 4.0 ** SQV)

        def nr_rsqrt(dst, src_ps, n, scale16):
            """dst[128, n] = (src_ps + 256*eps)^(-1/2) (*16), DVE only."""
            x = nr_p.tile([128, 8], F32, tag="nrx", name=f"nrx{nr_rsqrt.i}")[:, :n]
            nc.vector.tensor_scalar(out=x, in0=src_ps, scalar1=LN_EPS,
                                    scalar2=None, op0=mybir.AluOpType.add)
            s = nr_p.tile([128, 8], F32, tag="nry", name=f"nry{nr_rsqrt.i}")[:, :n]
            nc.vector.tensor_scalar(
                out=s.bitcast(I32), in0=x.bitcast(I32), scalar1=1,
                scalar2=None, op0=mybir.AluOpType.arith_shift_right)
            nc.vector.tensor_scalar(
                out=s.bitcast(I32), in0=s.bitcast(I32), scalar1=SQRT_MAGIC,
                scalar2=None, op0=mybir.AluOpType.add)
            for it in range(2):
                r = nr_p.tile([128, 8], F32, tag="nrt",
                              name=f"nrt{nr_rsqrt.i}_{it}")[:, :n]
                nc.vector.reciprocal(out=r, in_=s)
                nc.vector.tensor_mul(out=r, in0=r, in1=x)     # x / s
                nc.vector.tensor_add(out=s, in0=s, in1=r)
                nc.vector.tensor_scalar(out=s, in0=s, scalar1=0.5,
                                        scalar2=None, op0=mybir.AluOpType.mult)
            nr_rsqrt.i += 1
            if dst.dtype != F32:
                with nc.allow_low_precision(reason="per-position q scale"):
                    nc.vector.reciprocal(out=dst, in_=s)
            else:
                nc.vector.reciprocal(out=dst, in_=s)
            if scale16:
                nc.vector.tensor_scalar(out=dst, in0=dst, scalar1=16.0,
                                        scalar2=None, op0=mybir.AluOpType.mult)

        nr_rsqrt.i = 0

        def col_to_row(src, n, split=False):
            """[128, n] f32 cols -> [1, n*128] row on partition 0.

            row[0, 128*j + p] = src[p, j].  With split=True the per-column
            DMAs alternate Pool-SWDGE / Act-HWDGE queues so the serial chain
            halves (used for the final chunk, where it gates the tail)."""
            row = nr_p.tile([1, 512], F32, tag="row",
                            name=f"row{col_to_row.i}")
            col_to_row.i += 1
            for j in range(n):
                eng = nc.scalar if (split and j % 2) else nc.gpsimd
                eng.dma_start(
                    out=row[:, j * 128:(j + 1) * 128], in_=src[:, j:j + 1])
            return row
        col_to_row.i = 0

        def _rope(dst_sb, et0, x1, x2, tab):
            """dst[:, et0, :]   = x1*tab[0] - x2*tab[3]   (cw1, sw2)
               dst[:, et0+1, :] = x2*tab[2] + x1*tab[1]   (cw2, sw1)"""
            t1 = rtmp_p.tile([128, CH], BF16, tag="rtmp")
            t2 = rtmp_p.tile([128, CH], BF16, tag="rtmp")
            nc.vector.tensor_mul(out=t1, in0=x1, in1=tab[:, 0, :])
            nc.vector.tensor_mul(out=t2, in0=x2, in1=tab[:, 3, :])
            nc.vector.tensor_sub(out=dst_sb[:, et0, :], in0=t1, in1=t2)
            t3 = rtmp_p.tile([128, CH], BF16, tag="rtmp")
            t4 = rtmp_p.tile([128, CH], BF16, tag="rtmp")
            nc.vector.tensor_mul(out=t3, in0=x2, in1=tab[:, 2, :])
            nc.vector.tensor_mul(out=t4, in0=x1, in1=tab[:, 1, :])
            nc.vector.tensor_add(out=dst_sb[:, et0 + 1, :], in0=t3, in1=t4)

        def proj_mms(pss, w_t, xs, ets):
            """3-term fp8 hi/lo projection for the e-tiles in `ets`:
            pss[i][e,:] += W.T @ x over full D, dtp-outer so each x tile /
            weight slice is consumed right as its DMA lands (and the next
            chunk's x loads can start as soon as v frees a buffer).

            hi.hi via DoubleRow k-tile pairs, then per-k-tile cross passes
            (W slot dim is (lo,hi), x slot dim is (hi,lo))."""
            for dtp in range(DTP):
                for i, et in enumerate(ets):
                    ecols = slice(et * 128, (et + 1) * 128)
                    nc.tensor.matmul(
                        pss[i], w_t[dtp][:, :, 1, ecols], xs[dtp][:, :, 0, :],
                        start=(dtp == 0), stop=False, perf_mode=DR)
                for j in range(2):
                    for i, et in enumerate(ets):
                        ecols = slice(et * 128, (et + 1) * 128)
                        nc.tensor.matmul(
                            pss[i], w_t[dtp][:, j, :, ecols],
                            xs[dtp][:, j, :, :],
                            start=False, stop=(dtp == DTP - 1 and j == 1),
                            perf_mode=DR)

        def prefetch_x(c):
            """Emit chunk c's x loads (SP queue) ahead of any later store
            DMAs so they never queue behind phase_b/c output traffic."""
            cs = slice(c * CH, (c + 1) * CH)
            xs = []
            for dtp in range(DTP):
                t = xs_p.tile([128, 2, 2, CH], F8, tag="xs",
                              name=f"x{c}_{dtp}")
                nc.sync.dma_start(out=t, in_=x8_d[dtp, :, :, :, cs])
                xs.append(t)
            return xs

        def phase_a(c, xs, first=False):
            """QKV projections + rmsnorm + rope for chunk c."""
            cs = slice(c * CH, (c + 1) * CH)
            # chunk 0's tables ride the (idle-at-start, in-order) SP queue so
            # they don't jump DMA_ENGINES ahead of the first wq/x transfers;
            # later chunks use the Pool SWDGE queue, whose buffer-recycle wait
            # would otherwise block x prefetch on the SP ring.
            rt_eng = nc.gpsimd if c > 0 else nc.sync
            rtq_t = rt_p.tile([128, 4, CH], BF16, tag="rtq")
            rt_eng.dma_start(out=rtq_t,
                             in_=rtq_d[:, :, cs].rearrange("t p s -> p t s"))
            rtk_t = rt_p.tile([128, 4, CH], BF16, tag="rtk")
            rt_eng.dma_start(out=rtk_t,
                             in_=rtk_d[:, :, cs].rearrange("t p s -> p t s"))
            if first:
                load_wk()

            # ---------- q ----------
            rq_ps = ps_col.tile([128, 8], F32, tag="col")
            qr = []
            sqs = []
            for eth in range(2):
                ets = (2 * eth, 2 * eth + 1)
                pss = [ps_big.tile([128, CH], F32, tag="big",
                                   name=f"qps{c}_{et}") for et in ets]
                proj_mms(pss, wq_t, xs, ets)
                for q_ps in pss:
                    sqt = sq_p.tile([128, CH], BF16, tag="sq")
                    nc.scalar.activation(
                        out=sqt, in_=q_ps,
                        func=mybir.ActivationFunctionType.Square,
                        bias=0.0, scale=1.0)
                    sqs.append(sqt)
                    t = qr_p.tile([128, CH], BF16, tag="qr")
                    nc.scalar.copy(out=t, in_=q_ps)
                    qr.append(t)
            def q_colmms():
                for et in range(ET):
                    hh = et // 2
                    for j in range(4):
                        nc.tensor.matmul(
                            rq_ps[:, 4 * hh + j: 4 * hh + j + 1],
                            sqs[et][:, j * 128:(j + 1) * 128], ones_sb,
                            start=(et == 0 and j == 0),
                            stop=(et == ET - 1 and j == 3))
            q_colmms()

            def q_tail():
                # bf16 scale path: the q1/q2 multiplies then run in DVE 2x
                # mode (all-2-byte operands); ~4e-3 relative on a per-position
                # q scale is well inside the error budget
                rq_sb = nr_p.tile([128, 8], BF16, tag="rq")
                nr_rsqrt(rq_sb, rq_ps, 8, scale16=False)
                t_row = nr_p.tile([1, 1024], BF16, tag="rowq",
                                  name=f"rowq{c}")
                for j in range(8):
                    nc.gpsimd.dma_start(
                        out=t_row[:, j * 128:(j + 1) * 128],
                        in_=rq_sb[:, j:j + 1])
                for hh in range(2):
                    rbt = rb_p.tile([128, CH], BF16, tag="rb")
                    nc.gpsimd.partition_broadcast(
                        rbt, t_row[0:1, hh * CH:(hh + 1) * CH])
                    q1 = qs_p.tile([128, CH], BF16, tag="qs")
                    nc.vector.tensor_mul(out=q1, in0=qr[2 * hh], in1=rbt)
                    q2 = qs_p.tile([128, CH], BF16, tag="qs")
                    nc.vector.tensor_mul(out=q2, in0=qr[2 * hh + 1], in1=rbt)
                    _rope(qT_t[c], 2 * hh, q1, q2, rtq_t)
            if first:
                load_wv()

            # ---------- k ----------
            rk_ps = ps_col.tile([128, 8], F32, tag="col")
            kr = []
            ksqs = []
            for eth in range(2):
                ets = (2 * eth, 2 * eth + 1)
                pss = [ps_big.tile([128, CH], F32, tag="big",
                                   name=f"kps{c}_{et}") for et in ets]
                proj_mms(pss, wk_t, xs, ets)
                for k_ps in pss:
                    sqt = sq_p.tile([128, CH], BF16, tag="sq")
                    nc.scalar.activation(
                        out=sqt, in_=k_ps,
                        func=mybir.ActivationFunctionType.Square,
                        bias=0.0, scale=1.0)
                    ksqs.append(sqt)
                    t = qr_p.tile([128, CH], BF16, tag="qr")
                    nc.scalar.copy(out=t, in_=k_ps)
                    kr.append(t)

            def k_colmms():
                for et in range(ET):
                    hh = et // 2
                    for j in range(4):
                        nc.tensor.matmul(
                            rk_ps[:, 4 * hh + j: 4 * hh + j + 1],
                            ksqs[et][:, j * 128:(j + 1) * 128], ones_sb,
                            start=(et == 0 and j == 0),
                            stop=(et == ET - 1 and j == 3))
            k_colmms()

            def k_tail():
                nr_rsqrt(rk_sb[:, 0, 4 * c:4 * c + 4], rk_ps[:, 0:4], 4,
                         scale16=True)
                nr_rsqrt(rk_sb[:, 1, 4 * c:4 * c + 4], rk_ps[:, 4:8], 4,
                         scale16=True)
                for hh in range(2):
                    _rope(kT_t[c], 2 * hh, kr[2 * hh], kr[2 * hh + 1],
                          rtk_t)

            # ---------- v ----------  (x stationary, W moving; same 3-term)
            # dtp-outer so each xs tile dies after its own 12 matmuls and the
            # next chunk's x loads can start while v is still running.
            v_pss = [ps_big.tile([128, EL], F32, tag="big",
                                 name=f"vps{c}_{st}") for st in range(4)]
            for dtp in range(DTP):
                for st in range(4):
                    stc = slice(st * 128, (st + 1) * 128)
                    nc.tensor.matmul(
                        v_pss[st], xs[dtp][:, :, 0, stc],
                        wv_sb[:, dtp, :, 1, :],
                        start=(dtp == 0), stop=False, perf_mode=DR)
                for j in range(2):
                    for st in range(4):
                        stc = slice(st * 128, (st + 1) * 128)
                        nc.tensor.matmul(
                            v_pss[st], xs[dtp][:, j, :, stc],
                            wv_sb[:, dtp, j, :, :],
                            start=False, stop=(dtp == DTP - 1 and j == 1),
                            perf_mode=DR)
            for st in range(4):
                # v psum carries 2^SQV — rescale here so downstream is 1:1
                nc.scalar.mul(out=v_t[c][:, st, :], in_=v_pss[st],
                              mul=2.0 ** -SQV)
            q_tail()
            if first:
                load_rest()
            return k_tail

        def phase_b(c, hh, aT_t, filler=None):
            """Attention for (chunk c, head hh), software-pipelined: av/den
            matmuls for tile k are emitted after the scores matmuls for tile
            k+1 so the PE never waits in-order on exp(k)."""
            cs = slice(c * CH, (c + 1) * CH)
            n_sk = 4 * c + 4
            av_ps = {i: ps_big.tile([128, CH], F32, tag="big",
                                    name=f"av{c}_{hh}_{i}") for i in range(2)}
            den_ps = ps_col.tile([128, 4], F32, tag="col",
                                 name=f"den{c}_{hh}")

            def av_mms(e_t, skt, lo):
                first, last = (skt == 0), (skt == n_sk - 1)
                for half in range(2):
                    nc.tensor.matmul(
                        av_ps[half][:, lo:],
                        v_t[skt // 4][:, skt % 4, hh * 256 + half * 128:
                                      hh * 256 + (half + 1) * 128],
                        e_t[:, lo:], start=first, stop=last)

            def den_mms(e_t, skt, lo):
                first, last = (skt == 0), (skt == n_sk - 1)
                for j in range(lo // 128, 4):
                    nc.tensor.matmul(
                        den_ps[:, j:j + 1],
                        e_t[:, j * 128:(j + 1) * 128], ones_sb,
                        start=(first and j == 0), stop=(last and j == 3))

            def consume(e_t, skt, lo):
                av_mms(e_t, skt, lo)
                den_mms(e_t, skt, lo)

            from collections import deque
            pend = deque()
            for skt in range(n_sk):
                rel = skt - 4 * c
                # columns below 128*rel of a diagonal tile are fully masked
                lo = max(rel, 0) * 128
                sc_ps = ps_big.tile([128, CH], F32, tag="big",
                                    name=f"sc{c}_{hh}_{skt}")
                for half in range(2):
                    et = 2 * hh + half
                    nc.tensor.matmul(
                        sc_ps[:, lo:],
                        kT_t[skt // 4][:, et, (skt % 4) * 128:
                                       (skt % 4 + 1) * 128],
                        qT_t[c][:, et, lo:],
                        start=(half == 0), stop=(half == 1))
                e_t = e_p.tile([128, CH], BF16, tag="ep",
                               name=f"et{c}_{hh}_{skt}")
                nc.scalar.activation(out=e_t[:, lo:], in_=sc_ps[:, lo:],
                                     func=mybir.ActivationFunctionType.Exp,
                                     bias=0.0,
                                     scale=rk_sb[:, hh, skt:skt + 1])
                if rel >= 0:
                    nc.vector.tensor_mul(out=e_t[:, lo:], in0=e_t[:, lo:],
                                         in1=msk_sb[:, rel, lo:])
                # depth 4 for the first steps (the filler pattern keeps
                # those psum banks free): rides out the Act backlog from
                # phase_a's tail before the first exp result is needed
                if len(pend) >= (4 if skt < 4 else 3):
                    consume(*pend.popleft())
                if filler is not None:
                    for fn in next(filler, []) or []:
                        fn()
                pend.append((e_t, skt, lo))
            # drain dens first: their exps are already done, and finishing
            # den early lets the reciprocal/broadcast chain overlap the av
            # drain instead of following it
            for args in pend:
                den_mms(*args)
            while pend:
                av_mms(*pend.popleft())
            if filler is not None:
                # a filler longer than n_sk steps must still emit everything:
                # silently dropping bursts would corrupt the output
                for fns in filler:
                    for fn in fns:
                        fn()
            avs = []
            for half in range(2):
                t = qs_p.tile([128, CH], BF16, tag="qs",
                              name=f"avs{c}_{hh}_{half}")
                # fold the 2^SA fp8 prescale of aT into this copy
                nc.scalar.mul(out=t, in_=av_ps[half], mul=2.0 ** SA)
                avs.append(t)
            dinv = nr_p.tile([128, 4], F32, tag="dinv", name=f"dinv{c}_{hh}")
            nc.vector.reciprocal(out=dinv, in_=den_ps)
            t_row = col_to_row(dinv, 4, split=True)
            rd_t = rd_p.tile([128, CH], F32, tag="rd", name=f"rd{c}_{hh}")
            nc.gpsimd.partition_broadcast(rd_t, t_row[0:1, 0:CH])
            for half in range(2):
                full = qs_p.tile([128, CH], BF16, tag="qs",
                                 name=f"atf{c}_{hh}_{half}")
                nc.vector.tensor_mul(out=full, in0=avs[half], in1=rd_t)
                hi = aT_t[:, hh, half, 0, :]
                nc.vector.tensor_copy(out=hi, in_=full)
                nc.vector.tensor_sub(out=aT_t[:, hh, half, 1, :],
                                     in0=full, in1=hi)

        o2_tiles = {}

        def c_head_mms(o_ps, aT_t, dout, etp, start, stop):
            dcols = slice(dout * 128, (dout + 1) * 128)
            nc.tensor.matmul(
                o_ps, wo_sb[:, etp, :, 1, dcols], aT_t[:, etp, :, 0, :],
                start=start, stop=False, perf_mode=DR)
            for j in range(2):
                nc.tensor.matmul(
                    o_ps, wo_sb[:, etp, j, :, dcols], aT_t[:, etp, j, :, :],
                    start=False, stop=(stop and j == 1), perf_mode=DR)

        def c_burst(c, aT_t, dout):
            cs = slice(c * CH, (c + 1) * CH)
            o_ps = ps_big.tile([128, CH], F32, tag="big",
                               name=f"ops{c}_{dout}")
            for etp in range(ETP):
                c_head_mms(o_ps, aT_t, dout, etp,
                           start=(etp == 0), stop=(etp == ETP - 1))
            c_store(c, dout, o_ps)

        def c_store(c, dout, o_ps):
            cs = slice(c * CH, (c + 1) * CH)
            # output tiles pair up two dout blocks per store DMA (HWDGE issue
            # overhead dominates small stores)
            if dout % 2 == 0:
                o_t = o_p.tile([128, 2, CH], BF16, tag="op",
                               name=f"ot{c}_{dout // 2}")
                o2_tiles[(c, dout // 2)] = o_t
                nc.vector.tensor_scalar(out=o_t[:, 0, :], in0=o_ps,
                                        scalar1=2.0 ** -SOUT, scalar2=None,
                                        op0=mybir.AluOpType.mult)
            else:
                o_t = o2_tiles.pop((c, dout // 2))
                nc.scalar.mul(out=o_t[:, 1, :], in_=o_ps, mul=2.0 ** -SOUT)
                eng = nc.scalar if (dout // 2) % 2 == 0 else nc.sync
                eng.dma_start(
                    out=out_d[dout - 1:dout + 1, :, cs].rearrange(
                        "j p s -> p j s"),
                    in_=o_t)

        def phase_c(c, aT_t):
            for dout in range(DT):
                c_burst(c, aT_t, dout)

        def phase_c_waves(c, aT_t):
            """Like phase_c but per 4-dout wave all head-0 contraction
            passes go first: head 1's aT lands last (its softmax chain runs
            after the final av drain), so this keeps PE fed meanwhile."""
            for w0 in range(0, DT, 4):
                pss = []
                for dout in range(w0, w0 + 4):
                    o_ps = ps_big.tile([128, CH], F32, tag="big",
                                       name=f"ops{c}_{dout}")
                    pss.append(o_ps)
                    c_head_mms(o_ps, aT_t, dout, 0, start=True, stop=False)
                for i, dout in enumerate(range(w0, w0 + 4)):
                    c_head_mms(pss[i], aT_t, dout, 1, start=False, stop=True)
                    c_store(c, dout, pss[i])

        def c_filler(c, aT_t, lo, hi, n_steps,
                     pattern=(0, 0, 0, 2, 2, 1, 1, 1)):
            """Yield exactly <= n_steps burst lists covering douts [lo, hi);
            the host phase_b calls next() n_steps times, so every dout MUST
            land within that budget or it would be silently dropped."""
            douts = list(range(lo, hi))
            plan = []
            for n in pattern:
                if len(plan) >= n_steps:
                    break
                take, douts = douts[:n], douts[n:]
                plan.append(take)
            while douts and len(plan) < n_steps:
                plan.append([douts.pop(0)])
            if douts:
                plan[-1].extend(douts)
            assert sum(len(p) for p in plan) == hi - lo and len(plan) <= n_steps
            for p in plan:
                yield [lambda c=c, a=aT_t, d=d: c_burst(c, a, d) for d in p]

        # ---- schedule ----
        aT = {}
        xs0 = prefetch_x(0)
        kt0 = phase_a(0, xs0, first=True)
        xs1 = prefetch_x(1)
        kt0()
        kt1 = phase_a(1, xs1)
        xs2 = prefetch_x(2)
        kt1()
        aT[0] = at_p.tile([128, ETP, 2, 2, CH], F8, tag="atp", name="aT0")
        phase_b(0, 0, aT[0])
        phase_b(0, 1, aT[0])
        kt2 = phase_a(2, xs2)
        xs3 = prefetch_x(3)
        aT[1] = at_p.tile([128, ETP, 2, 2, CH], F8, tag="atp", name="aT1")
        phase_b(1, 0, aT[1], filler=c_filler(0, aT[0], 0, 8, 8))
        phase_b(1, 1, aT[1], filler=c_filler(0, aT[0], 8, 16, 8, pattern=(0, 0, 0, 0, 2, 2, 2, 2)))
        kt2()
        kt3 = phase_a(3, xs3)
        aT[2] = at_p.tile([128, ETP, 2, 2, CH], F8, tag="atp", name="aT2")
        phase_b(2, 0, aT[2], filler=c_filler(1, aT[1], 0, 8, 12))
        phase_b(2, 1, aT[2], filler=c_filler(1, aT[1], 8, 16, 12, pattern=(0, 0, 0, 0, 2, 2, 2, 2)))
        kt3()
        aT[3] = at_p.tile([128, ETP, 2, 2, CH], F8, tag="atp", name="aT3")
        def _delayed(gen, skip):
            for _ in range(skip):
                yield []
            yield from gen
        phase_b(3, 0, aT[3], filler=_delayed(c_filler(2, aT[2], 0, 8, 8), 12))
        phase_b(3, 1, aT[3])
        for _d in range(8, 16):
            c_burst(2, aT[2], _d)
        phase_c_waves(3, aT[3])


def _host_tables(position_ids, q_norm_w, k_norm_w):
    pos = np.asarray(position_ids).astype(np.float64)
    inv = 1.0 / (10000.0 ** (np.arange(0, HD, 2, dtype=np.float64) / HD))
    ang = pos[:, None] * inv[None, :]                      # [S, 128]
    sin = np.sin(ang).astype(np.float32)
    cos = np.cos(ang).astype(np.float32)

    def fold(w):
        w = np.asarray(w, np.float32)
        w1, w2 = w[:HALF], w[HALF:]
        return np.stack([
            (cos * w1[None, :]).T,      # cw1 [128, S]
            (sin * w1[None, :]).T,      # sw1
            (cos * w2[None, :]).T,      # cw2
            (sin * w2[None, :]).T,      # sw2
        ]).astype(NBF)
    return fold(q_norm_w), fold(k_norm_w)


def _host_masks():
    p = np.arange(HALF)[:, None]
    n = np.arange(CH)[None, :]
    return np.stack(
        [(rel * HALF + p <= n) for rel in range(4)]).astype(NBF)


def _split8(a, shift, slot0_hi):
    """bf16-rounded a * 2^shift -> fp8 (hi, lo) stacked on a new last-but-one
    axis, ordered (hi, lo) if slot0_hi else (lo, hi)."""
    s = a.astype(NBF).astype(np.float32) * (2.0 ** shift)
    hi = s.astype(NF8)
    lo = (s - hi.astype(np.float32)).astype(NF8)
    pair = (hi, lo) if slot0_hi else (lo, hi)
    return np.stack(pair, axis=-2)


def _pack_w(w, shift):
    """[D_contract, N] -> [DTP, 2, 128, 2(lo,hi), N] fp8."""
    k = w.shape[0]
    sp = _split8(w, shift, slot0_hi=False)          # [K, 2, N]
    return np.ascontiguousarray(
        sp.reshape(k // 256, 2, 128, 2, w.shape[1]))


def kernel(**inputs):
    x = np.asarray(inputs["x"], np.float32)
    Wq = np.asarray(inputs["Wq"], np.float32)
    Wk = np.asarray(inputs["Wk"], np.float32)
    Wv = np.asarray(inputs["Wv"], np.float32)
    Wo = np.asarray(inputs["Wo"], np.float32)
    rtq, rtk = _host_tables(inputs["position_ids"],
                            inputs["q_norm_w"], inputs["k_norm_w"])
    masks = _host_masks()

    if "nc" not in _CACHE:
        _CACHE["nc"] = _build()
    nc = _CACHE["nc"]

    x8 = [np.ascontiguousarray(
        _split8(x[b].T, SX, slot0_hi=True).reshape(DTP, 2, 128, 2, S)
        .transpose(0, 2, 1, 3, 4))
        for b in range(B)]

    in_maps = []
    for core in range(N_CORES):
        b, g = divmod(core, 4)
        es = slice(g * EL, (g + 1) * EL)
        in_maps.append({
            "x8": x8[b],
            "wq8": _pack_w(Wq[es, :].T, SW),
            "wk8": _pack_w(Wk[es, :].T, SW),
            "wv8": _pack_w(Wv[es, :].T, SW),
            "wo8": _pack_w(Wo[:, es].T, SWO),
            "rtq": rtq, "rtk": rtk, "masks": masks,
        })
    res = run_bass_kernel_spmd(nc, in_maps, core_ids=list(range(N_CORES)))
    out = np.zeros((B, S, D), np.float32)
    for core in range(N_CORES):
        b = core // 4
        out[b] += res.results[core]["outT"].reshape(D, S).T.astype(np.float32)
    return out

